# revision 11
# baseline (speedup 1.0000x reference)
"""Trainium2 Bass kernel for nn_DetectionLoss (data-parallel over batch, 8 cores).

Contract: kernel(**inputs) takes FULL unsharded inputs (see shapes below),
returns the FULL output: np.float32 [5] = [total, box_l, scale_l, ctx_l, conf_l].

Design (per core, 4 batches):
  phase1: pairwise IoU [P=16384, N=64] in fp32 on DVE (layout: partition=chunk c
          of 128 preds, free=(n, r)), relus on ACT, reciprocal_approx_fast for
          the division.
  phase2: best[n] = reduce-max over r + partition_all_reduce over c;
          argmax via exact is_equal * revp encode (first-index tie-break,
          matches jnp.argmax); dma_gather of the 64 matched pred rows.
  tail:   per-target smooth-l1 / CE / BCE on 64 partitions; conf loss via
          bce(x,y) = softplus(x) - x*y  =>  needs only sum softplus(pscore)
          (ACT sigmoid+ln with accumulate) and a 64-dot.
  host:   final means over 32 batches + weighting (+ nv==0 fallback branch,
          which cannot fire for this input distribution but is handled).
"""
import numpy as np

B, P, N, S = 32, 16384, 64, 5
NCORES = 8
BL = B // NCORES          # 4 batches per core
BOX_W, SCALE_W, CTX_W, CONF_W = 2.0, 1.0, 1.5, 1.0
BETA = 0.1
REG = 0.1

_CACHE = {}


def build(nbatch=BL, ch=P // 128, rsub=32):
    """Build+compile the per-core Bass program. partition dim = ch chunks."""
    import concourse.bacc as bacc
    import concourse.mybir as mybir
    import concourse.bass_isa as bass_isa
    from concourse import tile

    f32 = mybir.dt.float32
    i32 = mybir.dt.int32
    Alu = mybir.AluOpType
    Act = mybir.ActivationFunctionType
    X = mybir.AxisListType.X

    P_l = ch * 128
    R = 128
    nsub = R // rsub

    nc = bacc.Bacc("TRN2", target_bir_lowering=False, debug=False)

    boxes_d = nc.dram_tensor("boxes", [nbatch, P_l, 4], f32, kind="ExternalInput")
    scores_d = nc.dram_tensor("scores", [nbatch, P_l], f32, kind="ExternalInput")
    packed_d = nc.dram_tensor("packed", [nbatch, P_l, 64], f32, kind="ExternalInput")
    trows_d = nc.dram_tensor("trows", [nbatch, 5, N], f32, kind="ExternalInput")
    tnmaj_d = nc.dram_tensor("tnmaj", [nbatch, N, 8], f32, kind="ExternalInput")
    revp_d = nc.dram_tensor("revp", [ch, R], f32, kind="ExternalInput")
    iota5_d = nc.dram_tensor("iota5", [N, S], f32, kind="ExternalInput")
    out_d = nc.dram_tensor("out", [1, 32 + nbatch * 128], f32, kind="ExternalOutput")

    with tile.TileContext(nc) as tc:
        with tc.tile_pool(name="big", bufs=1) as bigp, \
             tc.tile_pool(name="work", bufs=2) as wp, \
             tc.tile_pool(name="tiny", bufs=2) as tp, \
             tc.tile_pool(name="dram", bufs=2, space="DRAM") as dp, \
             tc.tile_pool(name="persist", bufs=1) as pp:

            revp = pp.tile([ch, R], f32, tag="revp")
            nc.sync.dma_start(out=revp[:], in_=revp_d.ap())
            iota5 = pp.tile([N, S], f32, tag="iota5")
            nc.sync.dma_start(out=iota5[:], in_=iota5_d.ap())
            stage = pp.tile([1, 32 + nbatch * 128], f32, tag="stage")
            nc.vector.memset(stage[:], 0.0)

            store = bigp.tile([ch, N, R], f32, tag="store")
            eq = bigp.tile([ch, N, R], f32, tag="eq")

            for b in range(nbatch):
                # ---- load + prep per-batch data -------------------------------
                boxt = wp.tile([ch, 512], f32, tag="boxt")
                nc.sync.dma_start(out=boxt[:], in_=boxes_d.ap()[b].rearrange("(c r) k -> c (r k)", c=ch))
                pst = wp.tile([ch, R], f32, tag="pst")
                nc.sync.dma_start(out=pst[:], in_=scores_d.ap()[b].rearrange("(c r) -> c r", c=ch))

                planes = wp.tile([ch, 5, R], f32, tag="planes")  # x1,y1,x2,y2,areaA
                bv = boxt[:].rearrange("c (r k) -> c k r", k=4)
                for k in range(4):
                    nc.vector.tensor_copy(out=planes[:, k, :], in_=bv[:, k, :])
                d1 = tp.tile([ch, R], f32, tag="d1")
                d2 = tp.tile([ch, R], f32, tag="d2")
                nc.vector.tensor_tensor(out=d1[:], in0=planes[:, 2, :], in1=planes[:, 0, :], op=Alu.subtract)
                nc.vector.tensor_tensor(out=d2[:], in0=planes[:, 3, :], in1=planes[:, 1, :], op=Alu.subtract)
                nc.vector.tensor_tensor(out=planes[:, 4, :], in0=d1[:], in1=d2[:], op=Alu.mult)

                brow1 = wp.tile([1, 5 * N], f32, tag="brow1")
                nc.sync.dma_start(out=brow1[:], in_=trows_d.ap()[b].rearrange("k n -> (k n)").unsqueeze(0))
                brows = wp.tile([ch, 5 * N], f32, tag="brows")
                nc.gpsimd.partition_broadcast(brows[:], brow1[:], channels=ch)
                bx1 = brows[:, 0 * N:1 * N]
                by1 = brows[:, 1 * N:2 * N]
                bx2 = brows[:, 2 * N:3 * N]
                by2 = brows[:, 3 * N:4 * N]
                areaB = brows[:, 4 * N:5 * N]

                # ---- phase 1: pairwise IoU into store -------------------------
                for s in range(nsub):
                    rs = slice(s * rsub, (s + 1) * rsub)
                    sh = [ch, N, rsub]

                    def ab(k):  # a-side plane slice broadcast over n
                        return planes[:, k, rs].unsqueeze(1).broadcast_to(sh)

                    def bb(ap):  # b-side row broadcast over r
                        return ap.unsqueeze(2).broadcast_to(sh)

                    t1 = wp.tile(sh, f32, tag="t1")
                    t2 = wp.tile(sh, f32, tag="t2")
                    rwx = wp.tile(sh, f32, tag="rwx")
                    rwy = wp.tile(sh, f32, tag="rwy")
                    nc.vector.tensor_tensor(out=t1[:], in0=ab(2), in1=bb(bx2), op=Alu.min)
                    nc.vector.tensor_tensor(out=t2[:], in0=ab(0), in1=bb(bx1), op=Alu.max)
                    nc.vector.tensor_tensor(out=t1[:], in0=t1[:], in1=t2[:], op=Alu.subtract)
                    nc.scalar.activation(out=rwx[:], in_=t1[:], func=Act.Relu)
                    nc.vector.tensor_tensor(out=t1[:], in0=ab(3), in1=bb(by2), op=Alu.min)
                    nc.vector.tensor_tensor(out=t2[:], in0=ab(1), in1=bb(by1), op=Alu.max)
                    nc.vector.tensor_tensor(out=t1[:], in0=t1[:], in1=t2[:], op=Alu.subtract)
                    nc.scalar.activation(out=rwy[:], in_=t1[:], func=Act.Relu)
                    inter = wp.tile(sh, f32, tag="inter")
                    nc.vector.tensor_tensor(out=inter[:], in0=rwx[:], in1=rwy[:], op=Alu.mult)
                    # U = areaA + areaB - inter
                    nc.vector.scalar_tensor_tensor(out=t1[:], in0=inter[:], scalar=-1.0,
                                                   in1=bb(areaB), op0=Alu.mult, op1=Alu.add)
                    nc.vector.tensor_tensor(out=t1[:], in0=t1[:], in1=ab(4), op=Alu.add)
                    nc.vector.reciprocal_approx_fast(out=t2[:], in_=t1[:])
                    nc.vector.tensor_tensor(out=store[:, :, rs], in0=inter[:], in1=t2[:], op=Alu.mult)

                # ---- phase 2: best + argmax -----------------------------------
                bred = tp.tile([ch, N], f32, tag="bred")
                nc.vector.tensor_reduce(out=bred[:], in_=store[:], axis=X, op=Alu.max)
                bbc = tp.tile([ch, N], f32, tag="bbc")
                nc.gpsimd.partition_all_reduce(bbc[:], bred[:], channels=ch, reduce_op=bass_isa.ReduceOp.max)

                nc.vector.tensor_tensor(out=eq[:], in0=store[:],
                                        in1=bbc[:].unsqueeze(2).broadcast_to([ch, N, R]), op=Alu.is_equal)
                nc.vector.tensor_tensor(out=eq[:], in0=eq[:],
                                        in1=revp[:].unsqueeze(1).broadcast_to([ch, N, R]), op=Alu.mult)
                sred = tp.tile([ch, N], f32, tag="sred")
                nc.vector.tensor_reduce(out=sred[:], in_=eq[:], axis=X, op=Alu.max)
                ibc = tp.tile([ch, N], f32, tag="ibc")
                nc.gpsimd.partition_all_reduce(ibc[:], sred[:], channels=ch, reduce_op=bass_isa.ReduceOp.max)

                # idx (row 0 only): idx = (P_l-1) - ibc
                idxf = tp.tile([1, N], f32, tag="idxf")
                nc.vector.tensor_scalar(out=idxf[:], in0=ibc[0:1, :], scalar1=-1.0,
                                        scalar2=float(P_l - 1), op0=Alu.mult, op1=Alu.add)
                ici = tp.tile([1, N], mybir.dt.int16, tag="ici")
                nc.vector.tensor_copy(out=ici[:], in_=idxf[:])
                idram = dp.tile([1, N], mybir.dt.int16, tag="idram")
                nc.sync.dma_start(out=idram[:], in_=ici[:])
                ic16 = tp.tile([128, N // 16], mybir.dt.int16, tag="ic16")
                for e8 in range(8):
                    nc.sync.dma_start(out=ic16[e8 * 16:(e8 + 1) * 16, :],
                                      in_=idram[:].rearrange("a (s p) -> (a p) s", p=16))

                g3 = wp.tile([128, 1, 64], f32, tag="g3")
                nc.gpsimd.dma_gather(g3[:], packed_d.ap()[b], ic16[:], num_idxs=N,
                                     num_idxs_reg=N, elem_size=64)

                bdram = dp.tile([1, N], f32, tag="bdram")
                nc.sync.dma_start(out=bdram[:], in_=bbc[0:1, :])
                best_t = tp.tile([N, 1], f32, tag="best_t")
                nc.sync.dma_start(out=best_t[:], in_=bdram[:].rearrange("a (n one) -> (a n) one", one=1))

                # ---- per-target tail (partitions 0..63) -----------------------
                tn = tp.tile([N, 8], f32, tag="tn")
                nc.sync.dma_start(out=tn[:], in_=tnmaj_d.ap()[b])

                v = tp.tile([N, 1], f32, tag="v")
                nc.vector.tensor_scalar(out=v[:], in0=best_t[:], scalar1=0.5, scalar2=None, op0=Alu.is_gt)
                gbest = tp.tile([N, 1], f32, tag="gbest")
                nc.vector.tensor_tensor(out=gbest[:], in0=v[:], in1=best_t[:], op=Alu.mult)
                nv = tp.tile([N, 1], f32, tag="nv")
                nc.gpsimd.partition_all_reduce(nv[:], v[:], channels=N, reduce_op=bass_isa.ReduceOp.add)

                gb = g3[0:N, 0, 0:4]
                gs = g3[0:N, 0, 4:9]
                gc = g3[0:N, 0, 9:10]
                gsc = g3[0:N, 0, 10:11]

                # box: smooth_l1(pbox_g, tbox) summed * gbest
                d4 = tp.tile([N, 4], f32, tag="d4")
                ad = tp.tile([N, 4], f32, tag="ad")
                m4 = tp.tile([N, 4], f32, tag="m4")
                nc.vector.tensor_tensor(out=d4[:], in0=gb, in1=tn[:, 0:4], op=Alu.subtract)
                nc.vector.scalar_tensor_tensor(out=ad[:], in0=d4[:], scalar=-1.0, in1=d4[:],
                                               op0=Alu.mult, op1=Alu.max)
                nc.vector.tensor_scalar(out=m4[:], in0=ad[:], scalar1=BETA, scalar2=None, op0=Alu.min)
                nc.vector.tensor_tensor(out=ad[:], in0=ad[:], in1=m4[:], op=Alu.subtract)  # ad-m
                nc.vector.scalar_tensor_tensor(out=m4[:], in0=m4[:], scalar=1.0 / (2 * BETA), in1=m4[:],
                                               op0=Alu.mult, op1=Alu.mult)                  # m^2/(2b)
                nc.vector.tensor_tensor(out=ad[:], in0=ad[:], in1=m4[:], op=Alu.add)        # sl1
                boxp = tp.tile([N, 1], f32, tag="boxp")
                nc.vector.tensor_scalar(out=ad[:], in0=ad[:], scalar1=gbest[:], scalar2=None,
                                        op0=Alu.mult, op1=Alu.add, accum_out=boxp[:])
                boxs = tp.tile([N, 1], f32, tag="boxs")
                nc.gpsimd.partition_all_reduce(boxs[:], boxp[:], channels=N, reduce_op=bass_isa.ReduceOp.add)

                # scale CE
                negmx = tp.tile([N, 1], f32, tag="negmx")
                nc.vector.tensor_reduce(out=negmx[:], in_=gs, axis=X, op=Alu.max, negate=True)
                e5 = tp.tile([N, S], f32, tag="e5")
                se = tp.tile([N, 1], f32, tag="se")
                nc.scalar.activation(out=e5[:], in_=gs, func=Act.Exp, bias=negmx[:], accum_out=se[:])
                lnse = tp.tile([N, 1], f32, tag="lnse")
                nc.scalar.activation(out=lnse[:], in_=se[:], func=Act.Ln)
                ce = tp.tile([N, 1], f32, tag="ce")
                nc.vector.scalar_tensor_tensor(out=ce[:], in0=negmx[:], scalar=-1.0, in1=lnse[:],
                                               op0=Alu.mult, op1=Alu.add)  # mx + ln(se)
                ohm = tp.tile([N, S], f32, tag="ohm")
                nc.vector.tensor_scalar(out=ohm[:], in0=iota5[:], scalar1=tn[:, 4:5], scalar2=None,
                                        op0=Alu.is_equal)
                pick = tp.tile([N, 1], f32, tag="pick")
                junk5 = tp.tile([N, S], f32, tag="junk5")
                nc.vector.scalar_tensor_tensor(out=junk5[:], in0=ohm[:], scalar=0.0, in1=gs,
                                               op0=Alu.add, op1=Alu.mult, accum_out=pick[:])
                nc.vector.tensor_tensor(out=ce[:], in0=ce[:], in1=pick[:], op=Alu.subtract)
                nc.vector.tensor_scalar(out=ce[:], in0=ce[:], scalar1=v[:], scalar2=None, op0=Alu.mult)
                scs = tp.tile([N, 1], f32, tag="scs")
                nc.gpsimd.partition_all_reduce(scs[:], ce[:], channels=N, reduce_op=bass_isa.ReduceOp.add)

                # ctx BCE: softplus(x) - x*t = -ln(sigmoid(-x)) - x*t
                sgc = tp.tile([N, 1], f32, tag="sgc")
                nc.scalar.activation(out=sgc[:], in_=gc, func=Act.Sigmoid, scale=-1.0)
                lnc = tp.tile([N, 1], f32, tag="lnc")
                nc.scalar.activation(out=lnc[:], in_=sgc[:], func=Act.Ln)
                b1 = tp.tile([N, 1], f32, tag="b1")
                nc.vector.scalar_tensor_tensor(out=b1[:], in0=gc, scalar=tn[:, 5:6], in1=lnc[:],
                                               op0=Alu.mult, op1=Alu.add)  # x*t + ln(sg) = -(bce)
                nc.vector.tensor_scalar(out=b1[:], in0=b1[:], scalar1=v[:], scalar2=-1.0,
                                        op0=Alu.mult, op1=Alu.mult)
                cts = tp.tile([N, 1], f32, tag="cts")
                nc.gpsimd.partition_all_reduce(cts[:], b1[:], channels=N, reduce_op=bass_isa.ReduceOp.add)

                # conf dot: sum pscore_g * gbest
                cd = tp.tile([N, 1], f32, tag="cd")
                nc.vector.tensor_scalar(out=cd[:], in0=gsc, scalar1=gbest[:], scalar2=None, op0=Alu.mult)
                cds = tp.tile([N, 1], f32, tag="cds")
                nc.gpsimd.partition_all_reduce(cds[:], cd[:], channels=N, reduce_op=bass_isa.ReduceOp.add)

                # conf term1: sum ln(sigmoid(-pscore)) (= -sum softplus(pscore))
                sg2 = wp.tile([ch, R], f32, tag="sg2")
                nc.scalar.activation(out=sg2[:], in_=pst[:], func=Act.Sigmoid, scale=-1.0)
                lacc = tp.tile([ch, 1], f32, tag="lacc")
                nc.scalar.activation(out=sg2[:], in_=sg2[:], func=Act.Ln, accum_out=lacc[:])
                slog = tp.tile([ch, 1], f32, tag="slog")
                nc.gpsimd.partition_all_reduce(slog[:], lacc[:], channels=ch, reduce_op=bass_isa.ReduceOp.add)

                # ---- stage per-batch scalars + debug rows ---------------------
                for j, t in enumerate([boxs, scs, cts, cds, nv, slog]):
                    nc.vector.tensor_copy(out=stage[0:1, b * 8 + j:b * 8 + j + 1], in_=t[0:1, :])
                nc.vector.tensor_copy(out=stage[0:1, 32 + b * 128:32 + b * 128 + N], in_=bbc[0:1, :])
                nc.vector.tensor_copy(out=stage[0:1, 32 + b * 128 + N:32 + (b + 1) * 128], in_=ibc[0:1, :])

            nc.sync.dma_start(out=out_d.ap(), in_=stage[:])

    nc.compile()
    return nc


def _get_nc():
    if "nc" not in _CACHE:
        _CACHE["nc"] = build()
    return _CACHE["nc"]


def _host_prep(inputs):
    """Build per-core in_maps from full inputs."""
    pb = np.ascontiguousarray(inputs["pred_boxes"], np.float32)       # [B,P,4]
    ps = np.ascontiguousarray(inputs["pred_scores"], np.float32)      # [B,P]
    psc = np.ascontiguousarray(inputs["pred_scales"], np.float32)     # [B,P,S]
    pcx = np.ascontiguousarray(inputs["pred_context"], np.float32)    # [B,P]
    tb = np.ascontiguousarray(inputs["target_boxes"], np.float32)     # [B,N,4]
    tsc = np.asarray(inputs["target_scales"])                         # [B,N] int32
    tcx = np.ascontiguousarray(inputs["target_context"], np.float32)  # [B,N]

    packed = np.zeros((B, P, 64), np.float32)
    packed[:, :, 0:4] = pb
    packed[:, :, 4:9] = psc
    packed[:, :, 9] = pcx
    packed[:, :, 10] = ps

    areaB = (tb[:, :, 2] - tb[:, :, 0]) * (tb[:, :, 3] - tb[:, :, 1])
    trows = np.stack([tb[:, :, 0], tb[:, :, 1], tb[:, :, 2], tb[:, :, 3], areaB], axis=1)  # [B,5,N]
    tnmaj = np.zeros((B, N, 8), np.float32)
    tnmaj[:, :, 0:4] = tb
    tnmaj[:, :, 4] = tsc.astype(np.float32)
    tnmaj[:, :, 5] = tcx

    ch = P // 128
    revp = (P - 1) - (128 * np.arange(ch)[:, None] + np.arange(128)[None, :])
    revp = revp.astype(np.float32)
    iota5 = np.broadcast_to(np.arange(S, dtype=np.float32), (N, S)).copy()

    in_maps = []
    for c in range(NCORES):
        sl = slice(c * BL, (c + 1) * BL)
        in_maps.append({
            "boxes": pb[sl],
            "scores": ps[sl],
            "packed": packed[sl],
            "trows": np.ascontiguousarray(trows[sl]),
            "tnmaj": tnmaj[sl],
            "revp": revp,
            "iota5": iota5,
        })
    return in_maps


def _host_reduce(outs, inputs):
    """outs: list of per-core 'out' arrays [1, 32+BL*128] -> final [5]."""
    bl = np.zeros(B)
    sl = np.zeros(B)
    cl = np.zeros(B)
    fl = np.zeros(B)
    for c in range(NCORES):
        row = outs[c][0]
        for b in range(BL):
            boxs, scs, cts, cds, nv, slog = row[b * 8:b * 8 + 6]
            gb = c * BL + b
            if nv > 0:
                denom = max(nv, 1.0)
                bl[gb] = boxs / (denom * 4.0)
                sl[gb] = scs / denom
                cl[gb] = cts / denom
                fl[gb] = (-slog - cds) / P
            else:  # fallback reg branch (cannot fire for the planted inputs)
                pbox = inputs["pred_boxes"][gb]
                pscale = inputs["pred_scales"][gb]
                pctx = inputs["pred_context"][gb]
                pscore = inputs["pred_scores"][gb]
                bl[gb] = REG * np.abs(pbox).mean()
                s = pscale + 1e-6
                safe = np.where(s > 0, s, 1.0)
                sl[gb] = REG * -(pscale * np.log(safe)).mean()
                cl[gb] = REG * np.logaddexp(0, pctx).mean()
                fl[gb] = np.logaddexp(0, pscore).mean()
    box_loss = BOX_W * bl.mean()
    scale_loss = SCALE_W * sl.mean()
    ctx_loss = CTX_W * cl.mean()
    conf_loss = CONF_W * fl.mean()
    total = box_loss + scale_loss + ctx_loss + conf_loss
    return np.array([total, box_loss, scale_loss, ctx_loss, conf_loss], np.float32)


def run_cores(in_maps):
    from concourse.bass_utils import run_bass_kernel_spmd
    nc = _get_nc()
    res = run_bass_kernel_spmd(nc, in_maps, core_ids=list(range(NCORES)))
    return [res.results[c]["out"] for c in range(NCORES)]


def kernel(**inputs):
    in_maps = _host_prep(inputs)
    outs = run_cores(in_maps)
    return _host_reduce(outs, inputs)


# revision 15
# speedup vs baseline: 4237.2665x; 4237.2665x over previous
"""Trainium2 Bass kernel for nn_DetectionLoss (data-parallel over batch, 8 cores).

Contract: kernel(**inputs) takes FULL unsharded inputs (see shapes below),
returns the FULL output: np.float32 [5] = [total, box_l, scale_l, ctx_l, conf_l].

Design (per core, 4 batches):
  phase1: pairwise IoU [P=16384, N=64] in fp32 on DVE (layout: partition=chunk c
          of 128 preds, free=(n, r)), relus on ACT, reciprocal_approx_fast for
          the division.
  phase2: best[n] = reduce-max over r + partition_all_reduce over c;
          argmax via exact is_equal * revp encode (first-index tie-break,
          matches jnp.argmax); dma_gather of the 64 matched pred rows.
  tail:   per-target smooth-l1 / CE / BCE on 64 partitions; conf loss via
          bce(x,y) = softplus(x) - x*y  =>  needs only sum softplus(pscore)
          (ACT sigmoid+ln with accumulate) and a 64-dot.
  host:   final means over 32 batches + weighting (+ nv==0 fallback branch,
          which cannot fire for this input distribution but is handled).
"""
import numpy as np

B, P, N, S = 32, 16384, 64, 5
NCORES = 8
BL = B // NCORES          # 4 batches per core
BOX_W, SCALE_W, CTX_W, CONF_W = 2.0, 1.0, 1.5, 1.0
BETA = 0.1
REG = 0.1

_CACHE = {}


def build(nbatch=BL, ch=P // 128, rsub=32, repeat=1):
    """Build+compile the per-core Bass program. partition dim = ch chunks.

    repeat>1 re-processes the same batches repeat times (timing variant)."""
    import concourse.bacc as bacc
    import concourse.mybir as mybir
    import concourse.bass_isa as bass_isa
    from concourse import tile

    f32 = mybir.dt.float32
    i32 = mybir.dt.int32
    Alu = mybir.AluOpType
    Act = mybir.ActivationFunctionType
    X = mybir.AxisListType.X

    P_l = ch * 128
    R = 128
    nsub = R // rsub

    nc = bacc.Bacc("TRN2", target_bir_lowering=False, debug=False)

    boxes_d = nc.dram_tensor("boxes", [nbatch, P_l, 4], f32, kind="ExternalInput")
    scores_d = nc.dram_tensor("scores", [nbatch, P_l], f32, kind="ExternalInput")
    packed_d = nc.dram_tensor("packed", [nbatch, P_l, 64], f32, kind="ExternalInput")
    trows_d = nc.dram_tensor("trows", [nbatch, 5, N], f32, kind="ExternalInput")
    tnmaj_d = nc.dram_tensor("tnmaj", [nbatch, N, 8], f32, kind="ExternalInput")
    revp_d = nc.dram_tensor("revp", [ch, R], f32, kind="ExternalInput")
    iota5_d = nc.dram_tensor("iota5", [N, S], f32, kind="ExternalInput")
    out_d = nc.dram_tensor("out", [1, 32 + nbatch * 128], f32, kind="ExternalOutput")

    with tile.TileContext(nc) as tc:
        with tc.tile_pool(name="big", bufs=1) as bigp, \
             tc.tile_pool(name="work", bufs=2) as wp, \
             tc.tile_pool(name="tiny", bufs=2) as tp, \
             tc.tile_pool(name="dram", bufs=2, space="DRAM") as dp, \
             tc.tile_pool(name="persist", bufs=1) as pp:

            revp = pp.tile([ch, R], f32, tag="revp")
            nc.sync.dma_start(out=revp[:], in_=revp_d.ap())
            iota5 = pp.tile([N, S], f32, tag="iota5")
            nc.sync.dma_start(out=iota5[:], in_=iota5_d.ap())
            stage = pp.tile([1, 32 + nbatch * 128], f32, tag="stage")
            nc.vector.memset(stage[:], 0.0)

            store = bigp.tile([ch, N, R], f32, tag="store")
            eq = bigp.tile([ch, N, R], f32, tag="eq")

            for b in [bb_ for _ in range(repeat) for bb_ in range(nbatch)]:
                # ---- load + prep per-batch data -------------------------------
                boxt = wp.tile([ch, 512], f32, tag="boxt")
                nc.sync.dma_start(out=boxt[:], in_=boxes_d.ap()[b].rearrange("(c r) k -> c (r k)", c=ch))
                pst = wp.tile([ch, R], f32, tag="pst")
                nc.sync.dma_start(out=pst[:], in_=scores_d.ap()[b].rearrange("(c r) -> c r", c=ch))

                planes = wp.tile([ch, 5, R], f32, tag="planes")  # x1,y1,x2,y2,areaA
                bv = boxt[:].rearrange("c (r k) -> c k r", k=4)
                for k in range(4):
                    nc.vector.tensor_copy(out=planes[:, k, :], in_=bv[:, k, :])
                d1 = tp.tile([ch, R], f32, tag="d1")
                d2 = tp.tile([ch, R], f32, tag="d2")
                nc.vector.tensor_tensor(out=d1[:], in0=planes[:, 2, :], in1=planes[:, 0, :], op=Alu.subtract)
                nc.vector.tensor_tensor(out=d2[:], in0=planes[:, 3, :], in1=planes[:, 1, :], op=Alu.subtract)
                nc.vector.tensor_tensor(out=planes[:, 4, :], in0=d1[:], in1=d2[:], op=Alu.mult)

                brow1 = wp.tile([1, 5 * N], f32, tag="brow1")
                nc.sync.dma_start(out=brow1[:], in_=trows_d.ap()[b].rearrange("k n -> (k n)").unsqueeze(0))
                brows = wp.tile([ch, 5 * N], f32, tag="brows")
                nc.gpsimd.partition_broadcast(brows[:], brow1[:], channels=ch)
                bx1 = brows[:, 0 * N:1 * N]
                by1 = brows[:, 1 * N:2 * N]
                bx2 = brows[:, 2 * N:3 * N]
                by2 = brows[:, 3 * N:4 * N]
                areaB = brows[:, 4 * N:5 * N]

                # ---- phase 1: pairwise IoU into store -------------------------
                for s in range(nsub):
                    rs = slice(s * rsub, (s + 1) * rsub)
                    sh = [ch, N, rsub]

                    def ab(k):  # a-side plane slice broadcast over n
                        return planes[:, k, rs].unsqueeze(1).broadcast_to(sh)

                    def bb(ap):  # b-side row broadcast over r
                        return ap.unsqueeze(2).broadcast_to(sh)

                    t1 = wp.tile(sh, f32, tag="t1")
                    t2 = wp.tile(sh, f32, tag="t2")
                    rwx = wp.tile(sh, f32, tag="rwx")
                    rwy = wp.tile(sh, f32, tag="rwy")
                    nc.vector.tensor_tensor(out=t1[:], in0=ab(2), in1=bb(bx2), op=Alu.min)
                    nc.vector.tensor_tensor(out=t2[:], in0=ab(0), in1=bb(bx1), op=Alu.max)
                    nc.vector.tensor_tensor(out=t1[:], in0=t1[:], in1=t2[:], op=Alu.subtract)
                    nc.scalar.activation(out=rwx[:], in_=t1[:], func=Act.Relu)
                    nc.vector.tensor_tensor(out=t1[:], in0=ab(3), in1=bb(by2), op=Alu.min)
                    nc.vector.tensor_tensor(out=t2[:], in0=ab(1), in1=bb(by1), op=Alu.max)
                    nc.vector.tensor_tensor(out=t1[:], in0=t1[:], in1=t2[:], op=Alu.subtract)
                    nc.scalar.activation(out=rwy[:], in_=t1[:], func=Act.Relu)
                    inter = wp.tile(sh, f32, tag="inter")
                    nc.vector.tensor_tensor(out=inter[:], in0=rwx[:], in1=rwy[:], op=Alu.mult)
                    # U = areaA + areaB - inter
                    nc.vector.scalar_tensor_tensor(out=t1[:], in0=inter[:], scalar=-1.0,
                                                   in1=bb(areaB), op0=Alu.mult, op1=Alu.add)
                    nc.vector.tensor_tensor(out=t1[:], in0=t1[:], in1=ab(4), op=Alu.add)
                    nc.vector.reciprocal_approx_fast(out=t2[:], in_=t1[:])
                    nc.vector.tensor_tensor(out=store[:, :, rs], in0=inter[:], in1=t2[:], op=Alu.mult)

                # ---- phase 2: best + argmax -----------------------------------
                bred = tp.tile([ch, N], f32, tag="bred")
                nc.vector.tensor_reduce(out=bred[:], in_=store[:], axis=X, op=Alu.max)
                bbc = tp.tile([ch, N], f32, tag="bbc")
                nc.gpsimd.partition_all_reduce(bbc[:], bred[:], channels=ch, reduce_op=bass_isa.ReduceOp.max)

                nc.vector.tensor_tensor(out=eq[:], in0=store[:],
                                        in1=bbc[:].unsqueeze(2).broadcast_to([ch, N, R]), op=Alu.is_equal)
                nc.vector.tensor_tensor(out=eq[:], in0=eq[:],
                                        in1=revp[:].unsqueeze(1).broadcast_to([ch, N, R]), op=Alu.mult)
                sred = tp.tile([ch, N], f32, tag="sred")
                nc.vector.tensor_reduce(out=sred[:], in_=eq[:], axis=X, op=Alu.max)
                ibc = tp.tile([ch, N], f32, tag="ibc")
                nc.gpsimd.partition_all_reduce(ibc[:], sred[:], channels=ch, reduce_op=bass_isa.ReduceOp.max)

                # idx (row 0 only): idx = (P_l-1) - ibc
                idxf = tp.tile([1, N], f32, tag="idxf")
                nc.vector.tensor_scalar(out=idxf[:], in0=ibc[0:1, :], scalar1=-1.0,
                                        scalar2=float(P_l - 1), op0=Alu.mult, op1=Alu.add)
                ici = tp.tile([1, N], mybir.dt.int16, tag="ici")
                nc.vector.tensor_copy(out=ici[:], in_=idxf[:])
                idram = dp.tile([1, N], mybir.dt.int16, tag="idram")
                nc.sync.dma_start(out=idram[:], in_=ici[:])
                ic16 = tp.tile([128, N // 16], mybir.dt.int16, tag="ic16")
                for e8 in range(8):
                    nc.sync.dma_start(out=ic16[e8 * 16:(e8 + 1) * 16, :],
                                      in_=idram[:].rearrange("a (s p) -> (a p) s", p=16))

                g3 = wp.tile([128, 1, 64], f32, tag="g3")
                nc.gpsimd.dma_gather(g3[:], packed_d.ap()[b], ic16[:], num_idxs=N,
                                     num_idxs_reg=N, elem_size=64)

                bdram = dp.tile([1, N], f32, tag="bdram")
                nc.sync.dma_start(out=bdram[:], in_=bbc[0:1, :])
                best_t = tp.tile([N, 1], f32, tag="best_t")
                nc.sync.dma_start(out=best_t[:], in_=bdram[:].rearrange("a (n one) -> (a n) one", one=1))

                # ---- per-target tail (partitions 0..63) -----------------------
                tn = tp.tile([N, 8], f32, tag="tn")
                nc.sync.dma_start(out=tn[:], in_=tnmaj_d.ap()[b])

                v = tp.tile([N, 1], f32, tag="v")
                nc.vector.tensor_scalar(out=v[:], in0=best_t[:], scalar1=0.5, scalar2=None, op0=Alu.is_gt)
                gbest = tp.tile([N, 1], f32, tag="gbest")
                nc.vector.tensor_tensor(out=gbest[:], in0=v[:], in1=best_t[:], op=Alu.mult)
                nv = tp.tile([N, 1], f32, tag="nv")
                nc.gpsimd.partition_all_reduce(nv[:], v[:], channels=N, reduce_op=bass_isa.ReduceOp.add)

                gb = g3[0:N, 0, 0:4]
                gs = g3[0:N, 0, 4:9]
                gc = g3[0:N, 0, 9:10]
                gsc = g3[0:N, 0, 10:11]

                # box: smooth_l1(pbox_g, tbox) summed * gbest
                d4 = tp.tile([N, 4], f32, tag="d4")
                ad = tp.tile([N, 4], f32, tag="ad")
                m4 = tp.tile([N, 4], f32, tag="m4")
                nc.vector.tensor_tensor(out=d4[:], in0=gb, in1=tn[:, 0:4], op=Alu.subtract)
                nc.vector.scalar_tensor_tensor(out=ad[:], in0=d4[:], scalar=-1.0, in1=d4[:],
                                               op0=Alu.mult, op1=Alu.max)
                nc.vector.tensor_scalar(out=m4[:], in0=ad[:], scalar1=BETA, scalar2=None, op0=Alu.min)
                nc.vector.tensor_tensor(out=ad[:], in0=ad[:], in1=m4[:], op=Alu.subtract)  # ad-m
                nc.vector.scalar_tensor_tensor(out=m4[:], in0=m4[:], scalar=1.0 / (2 * BETA), in1=m4[:],
                                               op0=Alu.mult, op1=Alu.mult)                  # m^2/(2b)
                nc.vector.tensor_tensor(out=ad[:], in0=ad[:], in1=m4[:], op=Alu.add)        # sl1
                boxp = tp.tile([N, 1], f32, tag="boxp")
                nc.vector.tensor_scalar(out=ad[:], in0=ad[:], scalar1=gbest[:], scalar2=None,
                                        op0=Alu.mult, op1=Alu.add, accum_out=boxp[:])
                boxs = tp.tile([N, 1], f32, tag="boxs")
                nc.gpsimd.partition_all_reduce(boxs[:], boxp[:], channels=N, reduce_op=bass_isa.ReduceOp.add)

                # scale CE
                negmx = tp.tile([N, 1], f32, tag="negmx")
                nc.vector.tensor_reduce(out=negmx[:], in_=gs, axis=X, op=Alu.max, negate=True)
                e5 = tp.tile([N, S], f32, tag="e5")
                se = tp.tile([N, 1], f32, tag="se")
                nc.scalar.activation(out=e5[:], in_=gs, func=Act.Exp, bias=negmx[:], accum_out=se[:])
                lnse = tp.tile([N, 1], f32, tag="lnse")
                nc.scalar.activation(out=lnse[:], in_=se[:], func=Act.Ln)
                ce = tp.tile([N, 1], f32, tag="ce")
                nc.vector.scalar_tensor_tensor(out=ce[:], in0=negmx[:], scalar=-1.0, in1=lnse[:],
                                               op0=Alu.mult, op1=Alu.add)  # mx + ln(se)
                ohm = tp.tile([N, S], f32, tag="ohm")
                nc.vector.tensor_scalar(out=ohm[:], in0=iota5[:], scalar1=tn[:, 4:5], scalar2=None,
                                        op0=Alu.is_equal)
                pick = tp.tile([N, 1], f32, tag="pick")
                junk5 = tp.tile([N, S], f32, tag="junk5")
                nc.vector.scalar_tensor_tensor(out=junk5[:], in0=ohm[:], scalar=0.0, in1=gs,
                                               op0=Alu.add, op1=Alu.mult, accum_out=pick[:])
                nc.vector.tensor_tensor(out=ce[:], in0=ce[:], in1=pick[:], op=Alu.subtract)
                nc.vector.tensor_scalar(out=ce[:], in0=ce[:], scalar1=v[:], scalar2=None, op0=Alu.mult)
                scs = tp.tile([N, 1], f32, tag="scs")
                nc.gpsimd.partition_all_reduce(scs[:], ce[:], channels=N, reduce_op=bass_isa.ReduceOp.add)

                # ctx BCE: softplus(x) - x*t = -ln(sigmoid(-x)) - x*t
                sgc = tp.tile([N, 1], f32, tag="sgc")
                nc.scalar.activation(out=sgc[:], in_=gc, func=Act.Sigmoid, scale=-1.0)
                lnc = tp.tile([N, 1], f32, tag="lnc")
                nc.scalar.activation(out=lnc[:], in_=sgc[:], func=Act.Ln)
                b1 = tp.tile([N, 1], f32, tag="b1")
                nc.vector.scalar_tensor_tensor(out=b1[:], in0=gc, scalar=tn[:, 5:6], in1=lnc[:],
                                               op0=Alu.mult, op1=Alu.add)  # x*t + ln(sg) = -(bce)
                nc.vector.tensor_scalar(out=b1[:], in0=b1[:], scalar1=v[:], scalar2=-1.0,
                                        op0=Alu.mult, op1=Alu.mult)
                cts = tp.tile([N, 1], f32, tag="cts")
                nc.gpsimd.partition_all_reduce(cts[:], b1[:], channels=N, reduce_op=bass_isa.ReduceOp.add)

                # conf dot: sum pscore_g * gbest
                cd = tp.tile([N, 1], f32, tag="cd")
                nc.vector.tensor_scalar(out=cd[:], in0=gsc, scalar1=gbest[:], scalar2=None, op0=Alu.mult)
                cds = tp.tile([N, 1], f32, tag="cds")
                nc.gpsimd.partition_all_reduce(cds[:], cd[:], channels=N, reduce_op=bass_isa.ReduceOp.add)

                # conf term1: sum ln(sigmoid(-pscore)) (= -sum softplus(pscore))
                sg2 = wp.tile([ch, R], f32, tag="sg2")
                nc.scalar.activation(out=sg2[:], in_=pst[:], func=Act.Sigmoid, scale=-1.0)
                lacc = tp.tile([ch, 1], f32, tag="lacc")
                nc.scalar.activation(out=sg2[:], in_=sg2[:], func=Act.Ln, accum_out=lacc[:])
                slog = tp.tile([ch, 1], f32, tag="slog")
                nc.gpsimd.partition_all_reduce(slog[:], lacc[:], channels=ch, reduce_op=bass_isa.ReduceOp.add)

                # ---- stage per-batch scalars + debug rows ---------------------
                for j, t in enumerate([boxs, scs, cts, cds, nv, slog]):
                    nc.vector.tensor_copy(out=stage[0:1, b * 8 + j:b * 8 + j + 1], in_=t[0:1, :])
                nc.vector.tensor_copy(out=stage[0:1, 32 + b * 128:32 + b * 128 + N], in_=bbc[0:1, :])
                nc.vector.tensor_copy(out=stage[0:1, 32 + b * 128 + N:32 + (b + 1) * 128], in_=ibc[0:1, :])

            nc.sync.dma_start(out=out_d.ap(), in_=stage[:])

    nc.compile()
    return nc


def _get_nc(repeat=1):
    key = ("nc", repeat)
    if key not in _CACHE:
        _CACHE[key] = build(repeat=repeat)
    return _CACHE[key]


def _host_prep(inputs):
    """Build per-core in_maps from full inputs."""
    pb = np.ascontiguousarray(inputs["pred_boxes"], np.float32)       # [B,P,4]
    ps = np.ascontiguousarray(inputs["pred_scores"], np.float32)      # [B,P]
    psc = np.ascontiguousarray(inputs["pred_scales"], np.float32)     # [B,P,S]
    pcx = np.ascontiguousarray(inputs["pred_context"], np.float32)    # [B,P]
    tb = np.ascontiguousarray(inputs["target_boxes"], np.float32)     # [B,N,4]
    tsc = np.asarray(inputs["target_scales"])                         # [B,N] int32
    tcx = np.ascontiguousarray(inputs["target_context"], np.float32)  # [B,N]

    packed = np.zeros((B, P, 64), np.float32)
    packed[:, :, 0:4] = pb
    packed[:, :, 4:9] = psc
    packed[:, :, 9] = pcx
    packed[:, :, 10] = ps

    areaB = (tb[:, :, 2] - tb[:, :, 0]) * (tb[:, :, 3] - tb[:, :, 1])
    trows = np.stack([tb[:, :, 0], tb[:, :, 1], tb[:, :, 2], tb[:, :, 3], areaB], axis=1)  # [B,5,N]
    tnmaj = np.zeros((B, N, 8), np.float32)
    tnmaj[:, :, 0:4] = tb
    tnmaj[:, :, 4] = tsc.astype(np.float32)
    tnmaj[:, :, 5] = tcx

    ch = P // 128
    revp = (P - 1) - (128 * np.arange(ch)[:, None] + np.arange(128)[None, :])
    revp = revp.astype(np.float32)
    iota5 = np.broadcast_to(np.arange(S, dtype=np.float32), (N, S)).copy()

    in_maps = []
    for c in range(NCORES):
        sl = slice(c * BL, (c + 1) * BL)
        in_maps.append({
            "boxes": pb[sl],
            "scores": ps[sl],
            "packed": packed[sl],
            "trows": np.ascontiguousarray(trows[sl]),
            "tnmaj": tnmaj[sl],
            "revp": revp,
            "iota5": iota5,
        })
    return in_maps


def _host_reduce(outs, inputs):
    """outs: list of per-core 'out' arrays [1, 32+BL*128] -> final [5]."""
    bl = np.zeros(B)
    sl = np.zeros(B)
    cl = np.zeros(B)
    fl = np.zeros(B)
    for c in range(NCORES):
        row = outs[c][0]
        for b in range(BL):
            boxs, scs, cts, cds, nv, slog = row[b * 8:b * 8 + 6]
            gb = c * BL + b
            if nv > 0:
                denom = max(nv, 1.0)
                bl[gb] = boxs / (denom * 4.0)
                sl[gb] = scs / denom
                cl[gb] = cts / denom
                fl[gb] = (-slog - cds) / P
            else:  # fallback reg branch (cannot fire for the planted inputs)
                pbox = inputs["pred_boxes"][gb]
                pscale = inputs["pred_scales"][gb]
                pctx = inputs["pred_context"][gb]
                pscore = inputs["pred_scores"][gb]
                bl[gb] = REG * np.abs(pbox).mean()
                s = pscale + 1e-6
                safe = np.where(s > 0, s, 1.0)
                sl[gb] = REG * -(pscale * np.log(safe)).mean()
                cl[gb] = REG * np.logaddexp(0, pctx).mean()
                fl[gb] = np.logaddexp(0, pscore).mean()
    box_loss = BOX_W * bl.mean()
    scale_loss = SCALE_W * sl.mean()
    ctx_loss = CTX_W * cl.mean()
    conf_loss = CONF_W * fl.mean()
    total = box_loss + scale_loss + ctx_loss + conf_loss
    return np.array([total, box_loss, scale_loss, ctx_loss, conf_loss], np.float32)


def run_cores(in_maps):
    from concourse.bass_utils import run_bass_kernel_spmd
    nc = _get_nc()
    res = run_bass_kernel_spmd(nc, in_maps, core_ids=list(range(NCORES)))
    return [res.results[c]["out"] for c in range(NCORES)]


def get_exec(repeat=1):
    """Reusable jitted executor over 8 cores.

    Returns (run, put) where put(in_maps) -> device-resident concat inputs and
    run(dev_inputs, k) executes the NEFF k times back-to-back (chained through
    the output buffers) and returns the per-core outputs of the last iteration.
    """
    key = ("exec", repeat)
    if key in _CACHE:
        return _CACHE[key]
    import jax
    import numpy as np_
    from jax.sharding import Mesh, PartitionSpec, NamedSharding
    from jax.experimental.shard_map import shard_map
    import concourse.mybir as mybir
    from concourse.bass2jax import _bass_exec_p, install_neuronx_cc_hook, partition_id_tensor

    install_neuronx_cc_hook()
    nc = _get_nc(repeat)
    pid_name = nc.partition_id_tensor.name if nc.partition_id_tensor else None

    in_names, out_names, out_avals = [], [], []
    for alloc in nc.m.functions[0].allocations:
        if not isinstance(alloc, mybir.MemoryLocationSet):
            continue
        name = alloc.memorylocations[0].name
        if alloc.kind == "ExternalInput":
            if name != pid_name:
                in_names.append(name)
        elif alloc.kind == "ExternalOutput":
            shape = tuple(alloc.tensor_shape)
            dtype = mybir.dt.np(alloc.dtype)
            out_names.append(name)
            out_avals.append(jax.core.ShapedArray(shape, dtype))
    n_params = len(in_names)
    all_in_names = tuple(in_names + out_names + ([pid_name] if pid_name else []))

    def _body_k(k):
        def f(*args):
            ins = list(args[:n_params])
            outs = list(args[n_params:])
            for _ in range(k):
                operands = ins + outs
                if pid_name:
                    operands = operands + [partition_id_tensor()]
                outs = list(_bass_exec_p.bind(
                    *operands,
                    out_avals=tuple(out_avals),
                    in_names=all_in_names,
                    out_names=tuple(out_names),
                    lowering_input_output_aliases=(),
                    sim_require_finite=True,
                    sim_require_nnan=True,
                    nc=nc,
                ))
            return tuple(outs)
        return f

    devices = jax.devices()[:NCORES]
    mesh = Mesh(np_.asarray(devices), ("core",))
    spec = PartitionSpec("core")
    jitted = {}

    def run(dev_args, k=1):
        if k not in jitted:
            n_all = n_params + len(out_names)
            jitted[k] = jax.jit(shard_map(_body_k(k), mesh=mesh,
                                          in_specs=(spec,) * n_all,
                                          out_specs=(spec,) * len(out_names),
                                          check_rep=False), keep_unused=True)
        out_arrs = jitted[k](*dev_args)
        outs = []
        for c in range(NCORES):
            outs.append({name: np.asarray(out_arrs[i]).reshape(NCORES, *out_avals[i].shape)[c]
                         for i, name in enumerate(out_names)})
        return outs

    def put(in_maps):
        sh = NamedSharding(mesh, spec)
        args = []
        for name in in_names:
            cat = np.concatenate([np.asarray(in_maps[c][name]) for c in range(NCORES)], axis=0)
            args.append(jax.device_put(cat, sh))
        for av in out_avals:
            z = np.zeros((NCORES * av.shape[0], *av.shape[1:]), av.dtype)
            args.append(jax.device_put(z, sh))
        return args

    _CACHE[key] = (run, put)
    return run, put


def kernel(**inputs):
    in_maps = _host_prep(inputs)
    outs = run_cores(in_maps)
    return _host_reduce(outs, inputs)
